# revision 51
# baseline (speedup 1.0000x reference)
"""Trainium2 Bass kernel: frequency-domain regularized (Wiener) deconvolution.

Reference computation (B=16, T=8192, C=8, FIL=16):
    h  = fft(w_real + i*w_imag)            # (FIL, T)
    g  = conj(h) / (|h|^2 + s)             # (FIL, T)
    xf = fft(x, axis=T)                    # per (b, c) row
    y  = real(ifft(xf[:,None,:,:] * g[None,:,None,:]))
    out = y -> (B, T, FIL*C) + bias

Sharding: data-parallel over batch across 8 cores (2 batches/core); filter
params replicated.  FFTs are 4-step Cooley-Tukey matmuls on the PE array
(T = N2*N1, N2=128, N1=64; n = n1 + N1*n2, k = k2 + N2*k1):

  forward:  M1 (contract n2, fp32r) -> twiddle W^(n1 k2) (DVE real half,
            Pool imag half) -> PE transpose T1 (4 per PSUM bank, ACT evac)
            -> M2 (contract n1, stacked-complex K) -> Z0A [k1r;k1i|(b,c,k2)]
  filter:   H-path processed in f-QUARTERS pipelined through (ACT stage,
            Pool/DVE twiddle, T1H, M2H, Hs evac); G pipeline runs twice on
            partition-packed [128, 512] halves covering f {0-3,8-11} then
            {4-7,12-15}; assembled into stacked bf16 G1=[Gr;Gr], G3=[Gi;Gi]
  inverse:  per (b,f) unit: zt1=Z0A_b*G1f, zt3=Z0A_b*G3f on DVE (some zt3
            on Pool); bias folded into the k=0 bin of zt1 (tiny DVE op);
            stage-1 iFFT contracts k1 with the DATA as matmul weights
            (out partitions = k2) and the complex add fused into PSUM
            accumulation via two weight matrices c_M3/c_M3p -> no PE
            transpose, no DVE add, no swapped Z0B copy; ACT evacuates
            [k2 | (ri,n1',f,c)] bf16 per unit; M4 contracts k2 per (b,n1')
            with inverse twiddle folded into static bf16 weights.  Units run
            b-major so M4(b0) overlaps batch-1's unit pipeline.
"""
import sys

sys.path.insert(0, "/opt/trn_rl_repo")

import numpy as np


def _get_cc():
    import concourse.bacc as bacc
    import concourse.mybir as mybir
    import concourse.tile as tile
    return bacc, mybir, tile


class Cfg:
    def __init__(self, T=8192, N2=128, N1=64, BL=2, C=8, FIL=16):
        assert N1 * N2 == T
        self.T, self.N2, self.N1, self.BL, self.C, self.FIL = T, N2, N1, BL, C, FIL
        self.ROWS = BL * C
        self.FC = FIL * C


FULL = Cfg()


def host_consts(cfg):
    """Static (input-independent) weights, as fp32 numpy arrays."""
    T, N1, N2 = cfg.T, cfg.N1, cfg.N2
    f32 = np.float32
    cs = {}
    a2 = np.arange(N2)
    a1 = np.arange(N1)
    F2 = np.exp(-2j * np.pi * np.outer(a2, a2) / N2)        # [n2, k2]
    cs["c_F2r"] = F2.real.astype(f32)
    cs["c_F2i"] = F2.imag.astype(f32)
    cs["c_F2in"] = (-F2.imag).astype(f32)
    Tw = np.exp(-2j * np.pi * np.outer(a2, a1) / T)         # [k2, n1]
    cs["c_Twr"] = Tw.real.astype(f32)
    cs["c_Twi"] = Tw.imag.astype(f32)
    cs["c_Twin"] = (-Tw.imag).astype(f32)
    F1 = np.exp(-2j * np.pi * np.outer(a1, a1) / N1)        # [n1, k1]
    cs["c_M2"] = np.hstack([np.vstack([F1.real, -F1.imag]),
                            np.vstack([F1.imag, F1.real])]).astype(f32)
    Fb1 = np.exp(2j * np.pi * np.outer(a1, a1) / N1)        # [k1, n1']
    M3 = np.hstack([np.vstack([Fb1.real, -Fb1.imag]),
                    np.vstack([Fb1.imag, Fb1.real])]).astype(f32)
    cs["c_M3"] = M3
    # row-swapped/sign-flipped variant: with zt3 = [Zr*gi; Zi*gi] where
    # gi = Hi*r = -Im(G),  zt3^T @ c_M3p == zt2^T @ c_M3 for the old
    # zt2 = [Zi*gi; -Zr*gi] (stacked-swap complex-multiply half)
    cs["c_M3p"] = np.vstack([-M3[N1:], M3[:N1]]).astype(f32)
    # M4 per-n1' weights, inverse twiddle folded in:
    #   L_{n1'}[k2, n2'] = exp(+2j pi k2 n2'/N2) * exp(+2j pi n1' k2 / T) / T
    Fb2 = np.exp(2j * np.pi * np.outer(a2, a2) / N2)        # [k2, n2']
    ph = np.exp(2j * np.pi * np.outer(a1, a2) / T)          # [n1', k2]
    L = Fb2[None, :, :] * ph[:, :, None] / T                # [n1', k2, n2']
    Lr = L.real.transpose(1, 0, 2).reshape(N2, N1 * N2)     # [k2, (n1', n2')]
    Lin = (-L.imag).transpose(1, 0, 2).reshape(N2, N1 * N2)
    cs["c_L"] = np.concatenate([Lr, Lin], axis=1).astype(f32)  # [k2 | (ri, n1', n2')]
    cs["c_idb"] = np.eye(N2, dtype=f32)
    return cs


# f-quarters: packed-G half 0 covers quarters (0, 2) = f {0..3, 8..11}
QGROUPS = [(0, 2), (1, 3)]
FORDER = [0, 1, 2, 3, 8, 9, 10, 11, 4, 5, 6, 7, 12, 13, 14, 15]


def build_nc(cfg, debug_dumps=False):
    bacc, mybir, tile = _get_cc()
    F32, F32R, BF16 = mybir.dt.float32, mybir.dt.float32r, mybir.dt.bfloat16
    AL = mybir.AluOpType
    T, N1, N2, BL, C, FIL = cfg.T, cfg.N1, cfg.N2, cfg.BL, cfg.C, cfg.FIL
    ROWS, FC = cfg.ROWS, cfg.FC
    N1s = 2 * N1          # stacked (real; imag) partition dim = 128
    KF = FIL * N2         # H/G free size, (f, k2) order = 2048
    KH = KF // 2          # packed layout free size = 1024
    KQ = KF // 4          # one f-quarter = 512
    RN = ROWS * N2        # Z0 free size, (b, c, k2) order = 2048
    KB = C * N2           # per-(b,f) free size = 1024
    MCH = 512

    nc = bacc.Bacc("TRN2", debug=False)

    xs_d = nc.dram_tensor("xs", [BL, T, C], F32R, kind="ExternalInput")
    wr_d = nc.dram_tensor("wr", [FIL, T], F32R, kind="ExternalInput")
    wi_d = nc.dram_tensor("wi", [FIL, T], F32R, kind="ExternalInput")
    srepP_d = nc.dram_tensor("srepP", [N2, KH], F32, kind="ExternalInput")
    bk0_d = nc.dram_tensor("bk0", [1, FIL * BL * C], F32, kind="ExternalInput")
    cdef = [
        ("c_F2r", [N2, N2], F32R), ("c_F2i", [N2, N2], F32R), ("c_F2in", [N2, N2], F32R),
        ("c_Twr", [N2, N1], BF16), ("c_Twi", [N2, N1], BF16), ("c_Twin", [N2, N1], BF16),
        ("c_M2", [N1s, N1s], BF16), ("c_M3", [N1s, N1s], BF16),
        ("c_M3p", [N1s, N1s], BF16),
        ("c_L", [N2, 2 * N1 * N2], BF16),
        ("c_idb", [N2, N2], BF16),
    ]
    cd = {}
    for name, shape, dt_ in cdef:
        cd[name] = nc.dram_tensor(name, shape, dt_, kind="ExternalInput")
    out_d = nc.dram_tensor("out", [BL, T, FC], F32, kind="ExternalOutput")
    dbg = {}
    if debug_dumps:
        for nm, shape, ddt in [("dZ0A", [N1s, RN], BF16),
                               ("dG1", [N1s, KF], BF16), ("dG3", [N1s, KF], BF16),
                               ("dZT1", [N1s, KB], BF16), ("dZT3", [N1s, KB], BF16),
                               ("dHs", [N1s, KF], F32),
                               ("dDT", [N2, BL * 2 * N1 * FIL * C], BF16)]:
            dbg[nm] = nc.dram_tensor(nm, shape, ddt, kind="ExternalOutput")

    with tile.TileContext(nc) as tc:
        with tc.tile_pool(name="consts", bufs=1) as cpool, \
             tc.tile_pool(name="spec", bufs=1) as spool:
            ct = {}
            for name, shape, dt_ in cdef:
                t_ = cpool.tile(shape, dt_, tag=name)
                ct[name] = t_
            bk0 = cpool.tile([1, FIL * BL * C], F32, tag="bk0")
            srepP = cpool.tile([N2, KH], F32, tag="srepP")

            def load_consts(names):
                for name in names:
                    nc.sync.dma_start(out=ct[name], in_=cd[name].ap())

            Z0A = spool.tile([N1s, RN], BF16, tag="Z0A")   # [k1r;k1i | (b,c,k2)]
            G1 = spool.tile([N1s, KF], BF16, tag="G1")     # [Gr;Gr | (f,k2)]
            G3 = spool.tile([N1s, KF], BF16, tag="G3")     # [Gi;Gi | (f,k2)]
            BT = spool.tile([N1s, RN], BF16, tag="BT")     # [n1r;n1i | (b,c,k2)]

            # ============ H forward (f-quarters) + x forward, interleaved ===
            with tc.tile_pool(name="fh", bufs=1) as hp, \
                 tc.tile_pool(name="fx", bufs=1) as fp, \
                 tc.tile_pool(name="gp", bufs=1) as gp:
                xts = []
                for b in range(BL):
                    xt = fp.tile([N2, N1 * C], F32R, tag=f"xt{b}")
                    nc.sync.dma_start(
                        out=xt, in_=xs_d.ap()[b].rearrange("(p q) c -> p (q c)", p=N2))
                    xts.append(xt)
                load_consts(["c_F2r", "c_F2i", "c_F2in"])
                wtr = hp.tile([N2, FIL * N1], F32R, tag="wtr")
                wti = hp.tile([N2, FIL * N1], F32R, tag="wti")
                nc.sync.dma_start(out=wtr.rearrange("p (f n) -> p f n", f=FIL),
                                  in_=wr_d.ap().rearrange("f (p n) -> p f n", p=N2))
                nc.sync.dma_start(out=wti.rearrange("p (f n) -> p f n", f=FIL),
                                  in_=wi_d.ap().rearrange("f (p n) -> p f n", p=N2))
                load_consts(["c_Twr", "c_Twi", "c_Twin", "c_M2", "c_idb",
                             "c_M3", "c_M3p"])
                nc.sync.dma_start(out=bk0, in_=bk0_d.ap())
                nc.sync.dma_start(out=srepP, in_=srepP_d.ap())

                Q = FIL // 4
                Hsbb = hp.tile([N2, 2 * FIL * N1], BF16, tag="Hsbb")
                BHc = hp.tile([N2, FIL * 2 * N1], BF16, tag="BHc")
                Asbs = []
                with tc.tile_pool(name="fxp", bufs=1, space="PSUM") as fps, \
                     tc.tile_pool(name="fhp", bufs=1, space="PSUM") as hps:
                    # --- PE: x-M1 (both b) first, then H-M1 ---
                    xps = []
                    for b in range(BL):
                        ps = fps.tile([N2, 2 * N1 * C], F32, tag=f"Aps{b}")
                        for comp, w in ((0, "c_F2r"), (1, "c_F2i")):
                            for c0, c1 in chunks_of(N1 * C, MCH):
                                nc.tensor.matmul(
                                    ps[:, comp * N1 * C + c0: comp * N1 * C + c1],
                                    ct[w], xts[b][:, c0:c1], start=True, stop=True)
                        xps.append(ps)
                    hps_t = hps.tile([N2, 2 * FIL * N1], F32, tag="Hps")
                    for c0, c1 in chunks_of(FIL * N1, MCH):
                        nc.tensor.matmul(hps_t[:, c0:c1], ct["c_F2r"], wtr[:, c0:c1],
                                         start=True, stop=False)
                        nc.tensor.matmul(hps_t[:, c0:c1], ct["c_F2in"], wti[:, c0:c1],
                                         start=False, stop=True)
                        d0 = FIL * N1
                        nc.tensor.matmul(hps_t[:, d0 + c0:d0 + c1], ct["c_F2i"],
                                         wtr[:, c0:c1], start=True, stop=False)
                        nc.tensor.matmul(hps_t[:, d0 + c0:d0 + c1], ct["c_F2r"],
                                         wti[:, c0:c1], start=False, stop=True)
                    # --- ACT: stage PSUM -> SBUF bf16 (layout (m, c, n) for x) ---
                    for b in range(BL):
                        Asb = fp.tile([N2, 2 * N1 * C], BF16, tag=f"Asb{b}")
                        Asbs.append(Asb)
                        nc.scalar.copy(
                            out=Asb.rearrange("p (m c n) -> p m n c", m=2, c=C),
                            in_=xps[b].rearrange("p (m n c) -> p m n c", m=2, c=C))
                    for q in (0, 2, 1, 3):
                        for d0 in (0, FIL * N1):
                            nc.scalar.copy(
                                out=Hsbb[:, d0 + q * Q * N1: d0 + (q + 1) * Q * N1],
                                in_=hps_t[:, d0 + q * Q * N1: d0 + (q + 1) * Q * N1])
                # PSUM released; all twiddles run from SBUF bf16 on DVE
                # --- x twiddle (DVE, bf16 2x): free order (c, n1) ---
                Bcs = []
                for b in range(BL):
                    Bc = fp.tile([N2, 2 * N1 * C], BF16, tag=f"Bc{b}")
                    Bcs.append(Bc)
                    u = fp.tile([N2, N1 * C], BF16, tag="u")
                    v = fp.tile([N2, N1 * C], BF16, tag="v")
                    Asv = Asbs[b].rearrange("p (m c n) -> p m c n", m=2, c=C)

                    def bcx(w):
                        return ct[w][:, None, :].broadcast_to([N2, C, N1])

                    uv = u.rearrange("p (c n) -> p c n", c=C)
                    vv = v.rearrange("p (c n) -> p c n", c=C)
                    Bv = Bc.rearrange("p (c m n) -> p m c n", c=C, m=2)
                    nc.vector.tensor_tensor(out=uv, in0=Asv[:, 0], in1=bcx("c_Twr"),
                                            op=AL.mult)
                    nc.vector.tensor_tensor(out=vv, in0=Asv[:, 1], in1=bcx("c_Twin"),
                                            op=AL.mult)
                    nc.vector.tensor_tensor(out=Bv[:, 0], in0=uv, in1=vv, op=AL.add)
                    nc.vector.tensor_tensor(out=uv, in0=Asv[:, 0], in1=bcx("c_Twi"),
                                            op=AL.mult)
                    nc.vector.tensor_tensor(out=vv, in0=Asv[:, 1], in1=bcx("c_Twr"),
                                            op=AL.mult)
                    nc.vector.tensor_tensor(out=Bv[:, 1], in0=uv, in1=vv, op=AL.add)

                def qtw(q):
                    # H twiddle for quarter q (DVE, bf16 2x); free order (f, n)
                    fsl = slice(q * Q * N1, (q + 1) * Q * N1)
                    Asrq = Hsbb[:, :FIL * N1][:, fsl].rearrange(
                        "p (f n) -> p f n", f=Q)
                    Asiq = Hsbb[:, FIL * N1:][:, fsl].rearrange(
                        "p (f n) -> p f n", f=Q)

                    def bchq(w):
                        return ct[w][:, None, :].broadcast_to([N2, Q, N1])

                    uhq = hp.tile([N2, Q * N1], BF16, tag="uh")
                    vhq = hp.tile([N2, Q * N1], BF16, tag="vh")
                    uvq = uhq.rearrange("p (f n) -> p f n", f=Q)
                    vvq = vhq.rearrange("p (f n) -> p f n", f=Q)
                    BHq = BHc[:, 2 * q * Q * N1:2 * (q + 1) * Q * N1].rearrange(
                        "p (f m n) -> p f m n", f=Q, m=2)
                    nc.vector.tensor_tensor(out=uvq, in0=Asrq, in1=bchq("c_Twr"),
                                            op=AL.mult)
                    nc.vector.tensor_tensor(out=vvq, in0=Asiq, in1=bchq("c_Twin"),
                                            op=AL.mult)
                    nc.vector.tensor_tensor(out=BHq[:, :, 0, :], in0=uvq, in1=vvq,
                                            op=AL.add)
                    nc.vector.tensor_tensor(out=uvq, in0=Asrq, in1=bchq("c_Twi"),
                                            op=AL.mult)
                    nc.vector.tensor_tensor(out=vvq, in0=Asiq, in1=bchq("c_Twr"),
                                            op=AL.mult)
                    nc.vector.tensor_tensor(out=BHq[:, :, 1, :], in0=uvq, in1=vvq,
                                            op=AL.add)

                Hs = hp.tile([N1s, KF], F32, tag="Hs")

                def qT(q, t1hps, m2hps):
                    # T1H transposes + M2H + Hs evac for quarter q
                    tp = t1hps.tile([N1s, 4 * N2], BF16, tag="t1h")
                    for j in range(4):
                        f = 4 * q + j
                        nc.tensor.transpose(
                            tp[:, j * N2:(j + 1) * N2],
                            BHc[:, f * 2 * N1:(f + 1) * 2 * N1], ct["c_idb"])
                    BTHq = hp.tile([N1s, 4 * N2], BF16, tag="BTH")
                    nc.scalar.copy(out=BTHq, in_=tp)
                    psq = m2hps.tile([N1s, 4 * N2], F32, tag="m2h")
                    nc.tensor.matmul(psq, ct["c_M2"], BTHq, start=True, stop=True)
                    nc.scalar.copy(out=Hs[:, q * KQ:(q + 1) * KQ], in_=psq)

                ghp = {}

                def ghalf_pre(h):
                    qa, qb = QGROUPS[h]
                    HrP = gp.tile([N2, KQ], F32, tag=f"HrP{h}")
                    HiP = gp.tile([N2, KQ], F32, tag=f"HiP{h}")
                    ghp[h] = (HrP, HiP)
                    nc.sync.dma_start(out=HrP[:N1, :],
                                      in_=Hs[:N1, qa * KQ:(qa + 1) * KQ])
                    nc.sync.dma_start(out=HrP[N1:, :],
                                      in_=Hs[:N1, qb * KQ:(qb + 1) * KQ])
                    nc.sync.dma_start(out=HiP[:N1, :],
                                      in_=Hs[N1:, qa * KQ:(qa + 1) * KQ])
                    nc.sync.dma_start(out=HiP[N1:, :],
                                      in_=Hs[N1:, qb * KQ:(qb + 1) * KQ])

                def ghalf(h):
                    qa, qb = QGROUPS[h]
                    HrP, HiP = ghp[h]
                    sq1 = gp.tile([N2, KQ], F32, tag=f"sq1{h}")
                    sq2 = gp.tile([N2, KQ], F32, tag=f"sq2{h}")
                    nc.gpsimd.tensor_tensor(out=sq1, in0=HrP, in1=HrP, op=AL.mult)
                    nc.gpsimd.tensor_tensor(out=sq2, in0=HiP, in1=HiP, op=AL.mult)
                    nc.gpsimd.tensor_tensor(out=sq2, in0=sq1, in1=sq2, op=AL.add)
                    srp = srepP[:, h * KQ:(h + 1) * KQ]
                    nc.vector.tensor_tensor(out=sq2, in0=sq2, in1=srp, op=AL.add)
                    r = sq1
                    nc.vector.reciprocal(out=r, in_=sq2)
                    GrPb = gp.tile([N2, KQ], BF16, tag=f"GrPb{h}")
                    GiPb = gp.tile([N2, KQ], BF16, tag=f"GiPb{h}")
                    nc.vector.tensor_tensor(out=GrPb, in0=HrP, in1=r, op=AL.mult)
                    nc.vector.tensor_tensor(out=GiPb, in0=HiP, in1=r, op=AL.mult)
                    # unpack to stacked [Gr;Gr] / [gi;gi] with gi = Hi*r
                    # (the Im-G sign lives in c_M3p)
                    for (srct, dstt, eng) in ((GrPb, G1, nc.sync),
                                              (GiPb, G3, nc.scalar)):
                        eng.dma_start(out=dstt[:N1, qa * KQ:(qa + 1) * KQ],
                                      in_=srct[:N1, :])
                        eng.dma_start(out=dstt[:N1, qb * KQ:(qb + 1) * KQ],
                                      in_=srct[N1:, :])
                        eng.dma_start(out=dstt[N1:, qa * KQ:(qa + 1) * KQ],
                                      in_=srct[:N1, :])
                        eng.dma_start(out=dstt[N1:, qb * KQ:(qb + 1) * KQ],
                                      in_=srct[N1:, :])

                qtw(0)
                qtw(2)
                with tc.tile_pool(name="m2hp", bufs=2, space="PSUM") as m2hps, \
                     tc.tile_pool(name="t1hp", bufs=2, space="PSUM") as t1hps:
                    qT(0, t1hps, m2hps)
                    qT(2, t1hps, m2hps)
                    ghalf_pre(0)
                    qtw(1)
                    qtw(3)
                    qT(1, t1hps, m2hps)
                    qT(3, t1hps, m2hps)
                    ghalf_pre(1)
                # --- x T1 transposes (4 per bank) + evac, M2, Z0A ---
                with tc.tile_pool(name="t1p", bufs=2, space="PSUM") as t1ps, \
                     tc.tile_pool(name="m2p", bufs=1, space="PSUM") as m2ps:
                    for b in range(BL):
                        Bview = Bcs[b].rearrange("p (c mn) -> p c mn", c=C)
                        for qq in range(C // 4):
                            tp = t1ps.tile([N1s, 4 * N2], BF16, tag="t1")
                            for j in range(4):
                                c = 4 * qq + j
                                nc.tensor.transpose(tp[:, j * N2:(j + 1) * N2],
                                                    Bview[:, c, :], ct["c_idb"])
                            row = b * C + 4 * qq
                            nc.scalar.copy(out=BT[:, row * N2:(row + 4) * N2], in_=tp)
                    psx = m2ps.tile([N1s, RN], F32, tag="m2")
                    for c0, c1 in chunks_of(RN, MCH):
                        nc.tensor.matmul(psx[:, c0:c1], ct["c_M2"], BT[:, c0:c1],
                                         start=True, stop=True)
                    nc.vector.tensor_copy(out=Z0A, in_=psx)
                ghalf(0)
                ghalf(1)
                nc.sync.dma_start(out=ct["c_L"], in_=cd["c_L"].ap())

            if debug_dumps:
                nc.gpsimd.dma_start(out=dbg["dZ0A"].ap(), in_=Z0A)
                nc.gpsimd.dma_start(out=dbg["dG1"].ap(), in_=G1)
                nc.gpsimd.dma_start(out=dbg["dG3"].ap(), in_=G3)
                nc.sync.dma_start(out=dbg["dHs"].ap(), in_=Hs)


            # ================= inverse units + M4, b-major =================
            DT = spool.tile([N2, BL * 2 * N1 * FIL * C], BF16, tag="DT")
            dtv = DT.rearrange("p (b ri n1 f c) -> p b ri n1 f c",
                               b=BL, ri=2, n1=N1, f=FIL)
            dt4 = DT.rearrange("p (b ri n1 fc) -> p b ri n1 fc", b=BL, ri=2, n1=N1)
            bkv = bk0.rearrange("p (f b c) -> p f b c", f=FIL, b=BL)
            NB = 4   # n1' per M4 PSUM group (1 bank)
            with tc.tile_pool(name="zt", bufs=4) as ztp, \
                 tc.tile_pool(name="invp", bufs=3, space="PSUM") as ips, \
                 tc.tile_pool(name="yp", bufs=2, space="PSUM") as yps, \
                 tc.tile_pool(name="yev", bufs=6) as yp:
                for b in range(BL):
                    for fi, f in enumerate(FORDER):
                        zb = Z0A[:, b * KB:(b + 1) * KB].rearrange(
                            "p (c k) -> p c k", c=C)
                        g1 = G1[:, f * N2:(f + 1) * N2][:, None, :].broadcast_to(
                            [N1s, C, N2])
                        g3 = G3[:, f * N2:(f + 1) * N2][:, None, :].broadcast_to(
                            [N1s, C, N2])
                        zt1 = ztp.tile([N1s, KB], BF16, tag="zt1")
                        zt3 = ztp.tile([N1s, KB], BF16, tag="zt3")
                        z1v = zt1.rearrange("p (c k) -> p c k", c=C)
                        z3v = zt3.rearrange("p (c k) -> p c k", c=C)
                        nc.vector.tensor_tensor(out=z1v, in0=zb, in1=g1, op=AL.mult)
                        # bias into the k=0 bin (k1=0 real, k2=0) of zt1;
                        # issued before zt3 so PE can start on zt1 early
                        z1k0 = zt1.rearrange("p (c k) -> p c k", c=C)[0:1, :, 0]
                        nc.vector.tensor_tensor(out=z1k0, in0=z1k0,
                                                in1=bkv[0:1, f, b], op=AL.add)
                        meng = nc.gpsimd if fi % 2 == 1 else nc.vector
                        meng.tensor_tensor(out=z3v, in0=zb, in1=g3, op=AL.mult)
                        if debug_dumps and f == 0 and b == 0:
                            nc.sync.dma_start(out=dbg["dZT1"].ap(), in_=zt1)
                            nc.sync.dma_start(out=dbg["dZT3"].ap(), in_=zt3)
                        cps = ips.tile([N2, C * N1s], F32, tag="cps")
                        for c in range(C):
                            sl = cps[:, c * N1s:(c + 1) * N1s]
                            nc.tensor.matmul(sl, zt1[:, c * N2:(c + 1) * N2],
                                             ct["c_M3"], start=True, stop=False)
                            nc.tensor.matmul(sl, zt3[:, c * N2:(c + 1) * N2],
                                             ct["c_M3p"], start=False, stop=True)
                        cpv = cps.rearrange("p (c ri n1) -> p ri n1 c", c=C, ri=2)
                        if fi in (3, 7, 11):
                            nc.vector.tensor_copy(out=dtv[:, b, :, :, f, :], in_=cpv)
                        else:
                            nc.scalar.copy(out=dtv[:, b, :, :, f, :], in_=cpv)
                    if debug_dumps and b == 0:
                        nc.gpsimd.dma_start(out=dbg["dDT"].ap(), in_=DT)
                    # ---- M4 for this batch (overlaps next batch's units) ----
                    for g0 in range(0, N1, NB):
                        ypsum = yps.tile([N2, NB * FC], F32, tag="yps")
                        for j in range(NB):
                            n1p = g0 + j
                            lr = ct["c_L"][:, n1p * N2:(n1p + 1) * N2]
                            li = ct["c_L"][:, (N1 + n1p) * N2:(N1 + n1p + 1) * N2]
                            sl = ypsum[:, j * FC:(j + 1) * FC]
                            nc.tensor.matmul(sl, lr, dt4[:, b, 0, n1p, :],
                                             start=True, stop=False)
                            nc.tensor.matmul(sl, li, dt4[:, b, 1, n1p, :],
                                             start=False, stop=True)
                        yt = yp.tile([N2, NB * FC], F32, tag="yt")
                        if (g0 // NB) % 2 == 1:
                            nc.vector.tensor_copy(out=yt, in_=ypsum)
                        else:
                            nc.scalar.copy(out=yt, in_=ypsum)
                        nc.sync.dma_start(
                            out=out_d.ap()[b].rearrange(
                                "(n2 n1) fc -> n2 n1 fc", n1=N1)[:, g0:g0 + NB, :],
                            in_=yt.rearrange("p (j fc) -> p j fc", j=NB))

    nc.compile()
    return nc


def chunks_of(total, step):
    return [(c0, min(total, c0 + step)) for c0 in range(0, total, step)]


def host_inputs(cfg, x_sh, w_real, w_imag, s, b):
    """Build the per-core in_map (numpy) for one core's batch shard."""
    import ml_dtypes
    cs = host_consts(cfg)
    N1, N2, T, FIL, C, BL = cfg.N1, cfg.N2, cfg.T, cfg.FIL, cfg.C, cfg.BL
    KQ = FIL * N2 // 4
    f32 = np.float32
    # packed s matching QGROUPS: half h rows 0..63 = quarter 2h? see QGROUPS
    S = np.broadcast_to(np.asarray(s, f32).reshape(FIL, 1), (FIL, N2)).reshape(-1)
    halves = []
    for (qa, qb) in QGROUPS:
        halves.append(np.concatenate([
            np.broadcast_to(S[qa * KQ:(qa + 1) * KQ], (N1, KQ)),
            np.broadcast_to(S[qb * KQ:(qb + 1) * KQ], (N1, KQ))], axis=0))
    srepP = np.concatenate(halves, axis=1).astype(f32).copy()
    bf = np.asarray(b, f32).reshape(FIL, C)
    bk0 = np.broadcast_to((T * bf)[:, None, :], (FIL, BL, C)).reshape(1, -1)
    m = {
        "xs": np.ascontiguousarray(x_sh, dtype=f32),
        "wr": np.ascontiguousarray(w_real, dtype=f32),
        "wi": np.ascontiguousarray(w_imag, dtype=f32),
        "srepP": srepP,
        "bk0": bk0.astype(f32).copy(),
    }
    for k, v in cs.items():
        if k in ("c_L", "c_M2", "c_M3", "c_M3p", "c_idb", "c_Twr", "c_Twi",
                 "c_Twin"):
            m[k] = v.astype(ml_dtypes.bfloat16)
        else:
            m[k] = v
    return m


_NC_CACHE = {}


def kernel(x, w_real, w_imag, s, b):
    """Full-input entry point: shard over 8 cores, run, gather."""
    from concourse.bass_utils import run_bass_kernel_spmd
    cfg = FULL
    n_cores = 8
    key = "full"
    if key not in _NC_CACHE:
        _NC_CACHE[key] = build_nc(cfg)
    nc = _NC_CACHE[key]
    x = np.asarray(x, dtype=np.float32)
    w_real = np.asarray(w_real, dtype=np.float32)
    w_imag = np.asarray(w_imag, dtype=np.float32)
    s = np.asarray(s, dtype=np.float32)
    b = np.asarray(b, dtype=np.float32)
    in_maps = []
    for i in range(n_cores):
        x_sh = x[i * cfg.BL:(i + 1) * cfg.BL]
        in_maps.append(host_inputs(cfg, x_sh, w_real, w_imag, s, b))
    res = run_bass_kernel_spmd(nc, in_maps, core_ids=list(range(n_cores)))
    outs = [res.results[i]["out"] for i in range(n_cores)]
    return np.concatenate(outs, axis=0).astype(np.float32)


# revision 52
# speedup vs baseline: 1.0248x; 1.0248x over previous
"""Trainium2 Bass kernel: frequency-domain regularized (Wiener) deconvolution.

Reference computation (B=16, T=8192, C=8, FIL=16):
    h  = fft(w_real + i*w_imag)            # (FIL, T)
    g  = conj(h) / (|h|^2 + s)             # (FIL, T)
    xf = fft(x, axis=T)                    # per (b, c) row
    y  = real(ifft(xf[:,None,:,:] * g[None,:,None,:]))
    out = y -> (B, T, FIL*C) + bias

Sharding: data-parallel over batch across 8 cores (2 batches/core); filter
params replicated.  FFTs are 4-step Cooley-Tukey matmuls on the PE array
(T = N2*N1, N2=128, N1=64; n = n1 + N1*n2, k = k2 + N2*k1):

  forward:  M1 (contract n2, fp32r) -> twiddle W^(n1 k2) (DVE real half,
            Pool imag half) -> PE transpose T1 (4 per PSUM bank, ACT evac)
            -> M2 (contract n1, stacked-complex K) -> Z0A [k1r;k1i|(b,c,k2)]
  filter:   H-path processed in f-QUARTERS pipelined through (ACT stage,
            Pool/DVE twiddle, T1H, M2H, Hs evac); G pipeline runs twice on
            partition-packed [128, 512] halves covering f {0-3,8-11} then
            {4-7,12-15}; assembled into stacked bf16 G1=[Gr;Gr], G3=[Gi;Gi]
  inverse:  per (b,f) unit: zt1=Z0A_b*G1f, zt3=Z0A_b*G3f on DVE (some zt3
            on Pool); bias folded into the k=0 bin of zt1 (tiny DVE op);
            stage-1 iFFT contracts k1 with the DATA as matmul weights
            (out partitions = k2) and the complex add fused into PSUM
            accumulation via two weight matrices c_M3/c_M3p -> no PE
            transpose, no DVE add, no swapped Z0B copy; ACT evacuates
            [k2 | (ri,n1',f,c)] bf16 per unit; M4 contracts k2 per (b,n1')
            with inverse twiddle folded into static bf16 weights.  Units run
            b-major so M4(b0) overlaps batch-1's unit pipeline.
"""
import sys

sys.path.insert(0, "/opt/trn_rl_repo")

import numpy as np


def _get_cc():
    import concourse.bacc as bacc
    import concourse.mybir as mybir
    import concourse.tile as tile
    return bacc, mybir, tile


class Cfg:
    def __init__(self, T=8192, N2=128, N1=64, BL=2, C=8, FIL=16):
        assert N1 * N2 == T
        self.T, self.N2, self.N1, self.BL, self.C, self.FIL = T, N2, N1, BL, C, FIL
        self.ROWS = BL * C
        self.FC = FIL * C


FULL = Cfg()


def host_consts(cfg):
    """Static (input-independent) weights, as fp32 numpy arrays."""
    T, N1, N2 = cfg.T, cfg.N1, cfg.N2
    f32 = np.float32
    cs = {}
    a2 = np.arange(N2)
    a1 = np.arange(N1)
    F2 = np.exp(-2j * np.pi * np.outer(a2, a2) / N2)        # [n2, k2]
    cs["c_F2r"] = F2.real.astype(f32)
    cs["c_F2i"] = F2.imag.astype(f32)
    cs["c_F2in"] = (-F2.imag).astype(f32)
    Tw = np.exp(-2j * np.pi * np.outer(a2, a1) / T)         # [k2, n1]
    cs["c_Twr"] = Tw.real.astype(f32)
    cs["c_Twi"] = Tw.imag.astype(f32)
    cs["c_Twin"] = (-Tw.imag).astype(f32)
    F1 = np.exp(-2j * np.pi * np.outer(a1, a1) / N1)        # [n1, k1]
    cs["c_M2"] = np.hstack([np.vstack([F1.real, -F1.imag]),
                            np.vstack([F1.imag, F1.real])]).astype(f32)
    Fb1 = np.exp(2j * np.pi * np.outer(a1, a1) / N1)        # [k1, n1']
    M3 = np.hstack([np.vstack([Fb1.real, -Fb1.imag]),
                    np.vstack([Fb1.imag, Fb1.real])]).astype(f32)
    cs["c_M3"] = M3
    # row-swapped/sign-flipped variant: with zt3 = [Zr*gi; Zi*gi] where
    # gi = Hi*r = -Im(G),  zt3^T @ c_M3p == zt2^T @ c_M3 for the old
    # zt2 = [Zi*gi; -Zr*gi] (stacked-swap complex-multiply half)
    cs["c_M3p"] = np.vstack([-M3[N1:], M3[:N1]]).astype(f32)
    # M4 per-n1' weights, inverse twiddle folded in:
    #   L_{n1'}[k2, n2'] = exp(+2j pi k2 n2'/N2) * exp(+2j pi n1' k2 / T) / T
    Fb2 = np.exp(2j * np.pi * np.outer(a2, a2) / N2)        # [k2, n2']
    ph = np.exp(2j * np.pi * np.outer(a1, a2) / T)          # [n1', k2]
    L = Fb2[None, :, :] * ph[:, :, None] / T                # [n1', k2, n2']
    Lr = L.real.transpose(1, 0, 2).reshape(N2, N1 * N2)     # [k2, (n1', n2')]
    Lin = (-L.imag).transpose(1, 0, 2).reshape(N2, N1 * N2)
    cs["c_L"] = np.concatenate([Lr, Lin], axis=1).astype(f32)  # [k2 | (ri, n1', n2')]
    cs["c_idb"] = np.eye(N2, dtype=f32)
    return cs


# f-quarters: packed-G half 0 covers quarters (0, 2) = f {0..3, 8..11}
QGROUPS = [(0, 2), (1, 3)]
FORDER = [0, 1, 2, 3, 8, 9, 10, 11, 4, 5, 6, 7, 12, 13, 14, 15]


def build_nc(cfg, debug_dumps=False):
    bacc, mybir, tile = _get_cc()
    F32, F32R, BF16 = mybir.dt.float32, mybir.dt.float32r, mybir.dt.bfloat16
    AL = mybir.AluOpType
    T, N1, N2, BL, C, FIL = cfg.T, cfg.N1, cfg.N2, cfg.BL, cfg.C, cfg.FIL
    ROWS, FC = cfg.ROWS, cfg.FC
    N1s = 2 * N1          # stacked (real; imag) partition dim = 128
    KF = FIL * N2         # H/G free size, (f, k2) order = 2048
    KH = KF // 2          # packed layout free size = 1024
    KQ = KF // 4          # one f-quarter = 512
    RN = ROWS * N2        # Z0 free size, (b, c, k2) order = 2048
    KB = C * N2           # per-(b,f) free size = 1024
    MCH = 512

    nc = bacc.Bacc("TRN2", debug=False)

    xs_d = nc.dram_tensor("xs", [BL, T, C], F32R, kind="ExternalInput")
    wr_d = nc.dram_tensor("wr", [FIL, T], F32R, kind="ExternalInput")
    wi_d = nc.dram_tensor("wi", [FIL, T], F32R, kind="ExternalInput")
    srepP_d = nc.dram_tensor("srepP", [N2, KH], F32, kind="ExternalInput")
    bk0_d = nc.dram_tensor("bk0", [1, FIL * BL * C], F32, kind="ExternalInput")
    cdef = [
        ("c_F2r", [N2, N2], F32R), ("c_F2i", [N2, N2], F32R), ("c_F2in", [N2, N2], F32R),
        ("c_Twr", [N2, N1], BF16), ("c_Twi", [N2, N1], BF16), ("c_Twin", [N2, N1], BF16),
        ("c_M2", [N1s, N1s], BF16), ("c_M3", [N1s, N1s], BF16),
        ("c_M3p", [N1s, N1s], BF16),
        ("c_L", [N2, 2 * N1 * N2], BF16),
        ("c_idb", [N2, N2], BF16),
    ]
    cd = {}
    for name, shape, dt_ in cdef:
        cd[name] = nc.dram_tensor(name, shape, dt_, kind="ExternalInput")
    out_d = nc.dram_tensor("out", [BL, T, FC], F32, kind="ExternalOutput")
    dbg = {}
    if debug_dumps:
        for nm, shape, ddt in [("dZ0A", [N1s, RN], BF16),
                               ("dG1", [N1s, KF], BF16), ("dG3", [N1s, KF], BF16),
                               ("dZT1", [N1s, KB], BF16), ("dZT3", [N1s, KB], BF16),
                               ("dHs", [N1s, KF], F32),
                               ("dDT", [N2, BL * 2 * N1 * FIL * C], BF16)]:
            dbg[nm] = nc.dram_tensor(nm, shape, ddt, kind="ExternalOutput")

    with tile.TileContext(nc) as tc:
        with tc.tile_pool(name="consts", bufs=1) as cpool, \
             tc.tile_pool(name="spec", bufs=1) as spool:
            ct = {}
            for name, shape, dt_ in cdef:
                t_ = cpool.tile(shape, dt_, tag=name)
                ct[name] = t_
            bk0 = cpool.tile([1, FIL * BL * C], F32, tag="bk0")
            srepP = cpool.tile([N2, KH], F32, tag="srepP")

            def load_consts(names):
                for name in names:
                    nc.sync.dma_start(out=ct[name], in_=cd[name].ap())

            Z0A = spool.tile([N1s, RN], BF16, tag="Z0A")   # [k1r;k1i | (b,c,k2)]
            G1 = spool.tile([N1s, KF], BF16, tag="G1")     # [Gr;Gr | (f,k2)]
            G3 = spool.tile([N1s, KF], BF16, tag="G3")     # [Gi;Gi | (f,k2)]
            BT = spool.tile([N1s, RN], BF16, tag="BT")     # [n1r;n1i | (b,c,k2)]

            # ============ H forward (f-quarters) + x forward, interleaved ===
            with tc.tile_pool(name="fh", bufs=1) as hp, \
                 tc.tile_pool(name="fx", bufs=1) as fp, \
                 tc.tile_pool(name="gp", bufs=1) as gp:
                xts = []
                for b in range(BL):
                    xt = fp.tile([N2, N1 * C], F32R, tag=f"xt{b}")
                    nc.sync.dma_start(
                        out=xt, in_=xs_d.ap()[b].rearrange("(p q) c -> p (q c)", p=N2))
                    xts.append(xt)
                load_consts(["c_F2r", "c_F2i", "c_F2in"])
                wtr = hp.tile([N2, FIL * N1], F32R, tag="wtr")
                wti = hp.tile([N2, FIL * N1], F32R, tag="wti")
                nc.sync.dma_start(out=wtr.rearrange("p (f n) -> p f n", f=FIL),
                                  in_=wr_d.ap().rearrange("f (p n) -> p f n", p=N2))
                nc.sync.dma_start(out=wti.rearrange("p (f n) -> p f n", f=FIL),
                                  in_=wi_d.ap().rearrange("f (p n) -> p f n", p=N2))
                load_consts(["c_Twr", "c_Twi", "c_Twin", "c_M2", "c_idb",
                             "c_M3", "c_M3p"])
                nc.sync.dma_start(out=bk0, in_=bk0_d.ap())
                nc.sync.dma_start(out=srepP, in_=srepP_d.ap())

                Q = FIL // 4
                Hsbb = hp.tile([N2, 2 * FIL * N1], BF16, tag="Hsbb")
                BHc = hp.tile([N2, FIL * 2 * N1], BF16, tag="BHc")
                Asbs = []
                with tc.tile_pool(name="fxp", bufs=1, space="PSUM") as fps, \
                     tc.tile_pool(name="fhp", bufs=1, space="PSUM") as hps:
                    # --- PE: x-M1 (both b) first, then H-M1 ---
                    xps = []
                    for b in range(BL):
                        ps = fps.tile([N2, 2 * N1 * C], F32, tag=f"Aps{b}")
                        for comp, w in ((0, "c_F2r"), (1, "c_F2i")):
                            for c0, c1 in chunks_of(N1 * C, MCH):
                                nc.tensor.matmul(
                                    ps[:, comp * N1 * C + c0: comp * N1 * C + c1],
                                    ct[w], xts[b][:, c0:c1], start=True, stop=True)
                        xps.append(ps)
                    hps_t = hps.tile([N2, 2 * FIL * N1], F32, tag="Hps")
                    for c0, c1 in chunks_of(FIL * N1, MCH):
                        nc.tensor.matmul(hps_t[:, c0:c1], ct["c_F2r"], wtr[:, c0:c1],
                                         start=True, stop=False)
                        nc.tensor.matmul(hps_t[:, c0:c1], ct["c_F2in"], wti[:, c0:c1],
                                         start=False, stop=True)
                        d0 = FIL * N1
                        nc.tensor.matmul(hps_t[:, d0 + c0:d0 + c1], ct["c_F2i"],
                                         wtr[:, c0:c1], start=True, stop=False)
                        nc.tensor.matmul(hps_t[:, d0 + c0:d0 + c1], ct["c_F2r"],
                                         wti[:, c0:c1], start=False, stop=True)
                    # --- ACT: stage PSUM -> SBUF bf16 (layout (m, c, n) for x) ---
                    for b in range(BL):
                        Asb = fp.tile([N2, 2 * N1 * C], BF16, tag=f"Asb{b}")
                        Asbs.append(Asb)
                        nc.scalar.copy(
                            out=Asb.rearrange("p (m c n) -> p m n c", m=2, c=C),
                            in_=xps[b].rearrange("p (m n c) -> p m n c", m=2, c=C))
                    for q in (0, 2, 1, 3):
                        for d0 in (0, FIL * N1):
                            nc.scalar.copy(
                                out=Hsbb[:, d0 + q * Q * N1: d0 + (q + 1) * Q * N1],
                                in_=hps_t[:, d0 + q * Q * N1: d0 + (q + 1) * Q * N1])
                # PSUM released; all twiddles run from SBUF bf16 on DVE
                # --- x twiddle (DVE, bf16 2x): free order (c, n1) ---
                Bcs = []
                for b in range(BL):
                    Bc = fp.tile([N2, 2 * N1 * C], BF16, tag=f"Bc{b}")
                    Bcs.append(Bc)
                    u = fp.tile([N2, N1 * C], BF16, tag="u")
                    v = fp.tile([N2, N1 * C], BF16, tag="v")
                    Asv = Asbs[b].rearrange("p (m c n) -> p m c n", m=2, c=C)

                    def bcx(w):
                        return ct[w][:, None, :].broadcast_to([N2, C, N1])

                    uv = u.rearrange("p (c n) -> p c n", c=C)
                    vv = v.rearrange("p (c n) -> p c n", c=C)
                    Bv = Bc.rearrange("p (c m n) -> p m c n", c=C, m=2)
                    nc.vector.tensor_tensor(out=uv, in0=Asv[:, 0], in1=bcx("c_Twr"),
                                            op=AL.mult)
                    nc.vector.tensor_tensor(out=vv, in0=Asv[:, 1], in1=bcx("c_Twin"),
                                            op=AL.mult)
                    nc.vector.tensor_tensor(out=Bv[:, 0], in0=uv, in1=vv, op=AL.add)
                    nc.vector.tensor_tensor(out=uv, in0=Asv[:, 0], in1=bcx("c_Twi"),
                                            op=AL.mult)
                    nc.vector.tensor_tensor(out=vv, in0=Asv[:, 1], in1=bcx("c_Twr"),
                                            op=AL.mult)
                    nc.vector.tensor_tensor(out=Bv[:, 1], in0=uv, in1=vv, op=AL.add)

                def qtw(q):
                    # H twiddle for quarter q (DVE, bf16 2x); free order (f, n)
                    fsl = slice(q * Q * N1, (q + 1) * Q * N1)
                    Asrq = Hsbb[:, :FIL * N1][:, fsl].rearrange(
                        "p (f n) -> p f n", f=Q)
                    Asiq = Hsbb[:, FIL * N1:][:, fsl].rearrange(
                        "p (f n) -> p f n", f=Q)

                    def bchq(w):
                        return ct[w][:, None, :].broadcast_to([N2, Q, N1])

                    uhq = hp.tile([N2, Q * N1], BF16, tag="uh")
                    vhq = hp.tile([N2, Q * N1], BF16, tag="vh")
                    uvq = uhq.rearrange("p (f n) -> p f n", f=Q)
                    vvq = vhq.rearrange("p (f n) -> p f n", f=Q)
                    BHq = BHc[:, 2 * q * Q * N1:2 * (q + 1) * Q * N1].rearrange(
                        "p (f m n) -> p f m n", f=Q, m=2)
                    nc.vector.tensor_tensor(out=uvq, in0=Asrq, in1=bchq("c_Twr"),
                                            op=AL.mult)
                    nc.vector.tensor_tensor(out=vvq, in0=Asiq, in1=bchq("c_Twin"),
                                            op=AL.mult)
                    nc.vector.tensor_tensor(out=BHq[:, :, 0, :], in0=uvq, in1=vvq,
                                            op=AL.add)
                    nc.vector.tensor_tensor(out=uvq, in0=Asrq, in1=bchq("c_Twi"),
                                            op=AL.mult)
                    nc.vector.tensor_tensor(out=vvq, in0=Asiq, in1=bchq("c_Twr"),
                                            op=AL.mult)
                    nc.vector.tensor_tensor(out=BHq[:, :, 1, :], in0=uvq, in1=vvq,
                                            op=AL.add)

                Hs = hp.tile([N1s, KF], F32, tag="Hs")

                def qT(q, t1hps, m2hps):
                    # T1H transposes + M2H + Hs evac for quarter q
                    tp = t1hps.tile([N1s, 4 * N2], BF16, tag="t1h")
                    for j in range(4):
                        f = 4 * q + j
                        nc.tensor.transpose(
                            tp[:, j * N2:(j + 1) * N2],
                            BHc[:, f * 2 * N1:(f + 1) * 2 * N1], ct["c_idb"])
                    BTHq = hp.tile([N1s, 4 * N2], BF16, tag="BTH")
                    nc.scalar.copy(out=BTHq, in_=tp)
                    psq = m2hps.tile([N1s, 4 * N2], F32, tag="m2h")
                    nc.tensor.matmul(psq, ct["c_M2"], BTHq, start=True, stop=True)
                    nc.scalar.copy(out=Hs[:, q * KQ:(q + 1) * KQ], in_=psq)

                ghp = {}

                def ghalf_pre(h):
                    qa, qb = QGROUPS[h]
                    HrP = gp.tile([N2, KQ], F32, tag=f"HrP{h}")
                    HiP = gp.tile([N2, KQ], F32, tag=f"HiP{h}")
                    ghp[h] = (HrP, HiP)
                    nc.sync.dma_start(out=HrP[:N1, :],
                                      in_=Hs[:N1, qa * KQ:(qa + 1) * KQ])
                    nc.sync.dma_start(out=HrP[N1:, :],
                                      in_=Hs[:N1, qb * KQ:(qb + 1) * KQ])
                    nc.sync.dma_start(out=HiP[:N1, :],
                                      in_=Hs[N1:, qa * KQ:(qa + 1) * KQ])
                    nc.sync.dma_start(out=HiP[N1:, :],
                                      in_=Hs[N1:, qb * KQ:(qb + 1) * KQ])

                def ghalf(h):
                    qa, qb = QGROUPS[h]
                    HrP, HiP = ghp[h]
                    sq1 = gp.tile([N2, KQ], F32, tag=f"sq1{h}")
                    sq2 = gp.tile([N2, KQ], F32, tag=f"sq2{h}")
                    nc.gpsimd.tensor_tensor(out=sq1, in0=HrP, in1=HrP, op=AL.mult)
                    nc.gpsimd.tensor_tensor(out=sq2, in0=HiP, in1=HiP, op=AL.mult)
                    nc.gpsimd.tensor_tensor(out=sq2, in0=sq1, in1=sq2, op=AL.add)
                    srp = srepP[:, h * KQ:(h + 1) * KQ]
                    nc.vector.tensor_tensor(out=sq2, in0=sq2, in1=srp, op=AL.add)
                    r = sq1
                    nc.vector.reciprocal(out=r, in_=sq2)
                    GrPb = gp.tile([N2, KQ], BF16, tag=f"GrPb{h}")
                    GiPb = gp.tile([N2, KQ], BF16, tag=f"GiPb{h}")
                    nc.vector.tensor_tensor(out=GrPb, in0=HrP, in1=r, op=AL.mult)
                    nc.vector.tensor_tensor(out=GiPb, in0=HiP, in1=r, op=AL.mult)
                    # unpack to stacked [Gr;Gr] / [gi;gi] with gi = Hi*r
                    # (the Im-G sign lives in c_M3p)
                    for (srct, dstt, eng) in ((GrPb, G1, nc.sync),
                                              (GiPb, G3, nc.scalar)):
                        eng.dma_start(out=dstt[:N1, qa * KQ:(qa + 1) * KQ],
                                      in_=srct[:N1, :])
                        eng.dma_start(out=dstt[:N1, qb * KQ:(qb + 1) * KQ],
                                      in_=srct[N1:, :])
                        eng.dma_start(out=dstt[N1:, qa * KQ:(qa + 1) * KQ],
                                      in_=srct[:N1, :])
                        eng.dma_start(out=dstt[N1:, qb * KQ:(qb + 1) * KQ],
                                      in_=srct[N1:, :])

                qtw(0)
                qtw(2)
                with tc.tile_pool(name="m2hp", bufs=2, space="PSUM") as m2hps, \
                     tc.tile_pool(name="t1hp", bufs=2, space="PSUM") as t1hps:
                    qT(0, t1hps, m2hps)
                    qT(2, t1hps, m2hps)
                    ghalf_pre(0)
                    qtw(1)
                    qtw(3)
                    qT(1, t1hps, m2hps)
                    qT(3, t1hps, m2hps)
                    ghalf_pre(1)
                # --- x T1 transposes (4 per bank) + evac, M2, Z0A ---
                with tc.tile_pool(name="t1p", bufs=2, space="PSUM") as t1ps, \
                     tc.tile_pool(name="m2p", bufs=1, space="PSUM") as m2ps:
                    for b in range(BL):
                        Bview = Bcs[b].rearrange("p (c mn) -> p c mn", c=C)
                        for qq in range(C // 4):
                            tp = t1ps.tile([N1s, 4 * N2], BF16, tag="t1")
                            for j in range(4):
                                c = 4 * qq + j
                                nc.tensor.transpose(tp[:, j * N2:(j + 1) * N2],
                                                    Bview[:, c, :], ct["c_idb"])
                            row = b * C + 4 * qq
                            nc.scalar.copy(out=BT[:, row * N2:(row + 4) * N2], in_=tp)
                    psx = m2ps.tile([N1s, RN], F32, tag="m2")
                    for c0, c1 in chunks_of(RN, MCH):
                        nc.tensor.matmul(psx[:, c0:c1], ct["c_M2"], BT[:, c0:c1],
                                         start=True, stop=True)
                    nc.vector.tensor_copy(out=Z0A, in_=psx)
                ghalf(0)
                ghalf(1)
                nc.sync.dma_start(out=ct["c_L"], in_=cd["c_L"].ap())

            if debug_dumps:
                nc.gpsimd.dma_start(out=dbg["dZ0A"].ap(), in_=Z0A)
                nc.gpsimd.dma_start(out=dbg["dG1"].ap(), in_=G1)
                nc.gpsimd.dma_start(out=dbg["dG3"].ap(), in_=G3)
                nc.sync.dma_start(out=dbg["dHs"].ap(), in_=Hs)


            # ================= inverse units + M4, b-major =================
            DT = spool.tile([N2, BL * 2 * N1 * FIL * C], BF16, tag="DT")
            dtv = DT.rearrange("p (b ri n1 f c) -> p b ri n1 f c",
                               b=BL, ri=2, n1=N1, f=FIL)
            dt4 = DT.rearrange("p (b ri n1 fc) -> p b ri n1 fc", b=BL, ri=2, n1=N1)
            bkv = bk0.rearrange("p (f b c) -> p f b c", f=FIL, b=BL)
            NB = 4   # n1' per M4 PSUM group (1 bank)
            with tc.tile_pool(name="zt", bufs=4) as ztp, \
                 tc.tile_pool(name="invp", bufs=3, space="PSUM") as ips, \
                 tc.tile_pool(name="yp", bufs=2, space="PSUM") as yps, \
                 tc.tile_pool(name="yev", bufs=6) as yp:
                for b in range(BL):
                    for fi, f in enumerate(FORDER):
                        zb = Z0A[:, b * KB:(b + 1) * KB].rearrange(
                            "p (c k) -> p c k", c=C)
                        g1 = G1[:, f * N2:(f + 1) * N2][:, None, :].broadcast_to(
                            [N1s, C, N2])
                        g3 = G3[:, f * N2:(f + 1) * N2][:, None, :].broadcast_to(
                            [N1s, C, N2])
                        zt1 = ztp.tile([N1s, KB], BF16, tag="zt1")
                        zt3 = ztp.tile([N1s, KB], BF16, tag="zt3")
                        z1v = zt1.rearrange("p (c k) -> p c k", c=C)
                        z3v = zt3.rearrange("p (c k) -> p c k", c=C)
                        nc.vector.tensor_tensor(out=z1v, in0=zb, in1=g1, op=AL.mult)
                        # bias into the k=0 bin (k1=0 real, k2=0) of zt1;
                        # issued before zt3 so PE can start on zt1 early
                        z1k0 = zt1.rearrange("p (c k) -> p c k", c=C)[0:1, :, 0]
                        nc.vector.tensor_tensor(out=z1k0, in0=z1k0,
                                                in1=bkv[0:1, f, b], op=AL.add)
                        meng = nc.gpsimd if fi % 2 == 1 else nc.vector
                        meng.tensor_tensor(out=z3v, in0=zb, in1=g3, op=AL.mult)
                        if debug_dumps and f == 0 and b == 0:
                            nc.sync.dma_start(out=dbg["dZT1"].ap(), in_=zt1)
                            nc.sync.dma_start(out=dbg["dZT3"].ap(), in_=zt3)
                        cps = ips.tile([N2, C * N1s], F32, tag="cps")
                        for c in range(C):
                            sl = cps[:, c * N1s:(c + 1) * N1s]
                            nc.tensor.matmul(sl, zt1[:, c * N2:(c + 1) * N2],
                                             ct["c_M3"], start=True, stop=False)
                            nc.tensor.matmul(sl, zt3[:, c * N2:(c + 1) * N2],
                                             ct["c_M3p"], start=False, stop=True)
                        cpv = cps.rearrange("p (c ri n1) -> p ri n1 c", c=C, ri=2)
                        if fi in (5, 11):
                            nc.vector.tensor_copy(out=dtv[:, b, :, :, f, :], in_=cpv)
                        else:
                            nc.scalar.copy(out=dtv[:, b, :, :, f, :], in_=cpv)
                    if debug_dumps and b == 0:
                        nc.gpsimd.dma_start(out=dbg["dDT"].ap(), in_=DT)
                    # ---- M4 for this batch (overlaps next batch's units) ----
                    for g0 in range(0, N1, NB):
                        ypsum = yps.tile([N2, NB * FC], F32, tag="yps")
                        for j in range(NB):
                            n1p = g0 + j
                            lr = ct["c_L"][:, n1p * N2:(n1p + 1) * N2]
                            li = ct["c_L"][:, (N1 + n1p) * N2:(N1 + n1p + 1) * N2]
                            sl = ypsum[:, j * FC:(j + 1) * FC]
                            nc.tensor.matmul(sl, lr, dt4[:, b, 0, n1p, :],
                                             start=True, stop=False)
                            nc.tensor.matmul(sl, li, dt4[:, b, 1, n1p, :],
                                             start=False, stop=True)
                        yt = yp.tile([N2, NB * FC], F32, tag="yt")
                        if (g0 // NB) % 2 == 1:
                            nc.vector.tensor_copy(out=yt, in_=ypsum)
                        else:
                            nc.scalar.copy(out=yt, in_=ypsum)
                        nc.sync.dma_start(
                            out=out_d.ap()[b].rearrange(
                                "(n2 n1) fc -> n2 n1 fc", n1=N1)[:, g0:g0 + NB, :],
                            in_=yt.rearrange("p (j fc) -> p j fc", j=NB))

    nc.compile()
    return nc


def chunks_of(total, step):
    return [(c0, min(total, c0 + step)) for c0 in range(0, total, step)]


def host_inputs(cfg, x_sh, w_real, w_imag, s, b):
    """Build the per-core in_map (numpy) for one core's batch shard."""
    import ml_dtypes
    cs = host_consts(cfg)
    N1, N2, T, FIL, C, BL = cfg.N1, cfg.N2, cfg.T, cfg.FIL, cfg.C, cfg.BL
    KQ = FIL * N2 // 4
    f32 = np.float32
    # packed s matching QGROUPS: half h rows 0..63 = quarter 2h? see QGROUPS
    S = np.broadcast_to(np.asarray(s, f32).reshape(FIL, 1), (FIL, N2)).reshape(-1)
    halves = []
    for (qa, qb) in QGROUPS:
        halves.append(np.concatenate([
            np.broadcast_to(S[qa * KQ:(qa + 1) * KQ], (N1, KQ)),
            np.broadcast_to(S[qb * KQ:(qb + 1) * KQ], (N1, KQ))], axis=0))
    srepP = np.concatenate(halves, axis=1).astype(f32).copy()
    bf = np.asarray(b, f32).reshape(FIL, C)
    bk0 = np.broadcast_to((T * bf)[:, None, :], (FIL, BL, C)).reshape(1, -1)
    m = {
        "xs": np.ascontiguousarray(x_sh, dtype=f32),
        "wr": np.ascontiguousarray(w_real, dtype=f32),
        "wi": np.ascontiguousarray(w_imag, dtype=f32),
        "srepP": srepP,
        "bk0": bk0.astype(f32).copy(),
    }
    for k, v in cs.items():
        if k in ("c_L", "c_M2", "c_M3", "c_M3p", "c_idb", "c_Twr", "c_Twi",
                 "c_Twin"):
            m[k] = v.astype(ml_dtypes.bfloat16)
        else:
            m[k] = v
    return m


_NC_CACHE = {}


def kernel(x, w_real, w_imag, s, b):
    """Full-input entry point: shard over 8 cores, run, gather."""
    from concourse.bass_utils import run_bass_kernel_spmd
    cfg = FULL
    n_cores = 8
    key = "full"
    if key not in _NC_CACHE:
        _NC_CACHE[key] = build_nc(cfg)
    nc = _NC_CACHE[key]
    x = np.asarray(x, dtype=np.float32)
    w_real = np.asarray(w_real, dtype=np.float32)
    w_imag = np.asarray(w_imag, dtype=np.float32)
    s = np.asarray(s, dtype=np.float32)
    b = np.asarray(b, dtype=np.float32)
    in_maps = []
    for i in range(n_cores):
        x_sh = x[i * cfg.BL:(i + 1) * cfg.BL]
        in_maps.append(host_inputs(cfg, x_sh, w_real, w_imag, s, b))
    res = run_bass_kernel_spmd(nc, in_maps, core_ids=list(range(n_cores)))
    outs = [res.results[i]["out"] for i in range(n_cores)]
    return np.concatenate(outs, axis=0).astype(np.float32)
